# revision 49
# baseline (speedup 1.0000x reference)
"""GCN layer on 8 Trainium2 NeuronCores.

out = D^-1/2 A D^-1/2 (values @ W + b),  A: [8192, 8192] f32 dense.

Strategy (row-parallel, host-interleaved slabs, 3-phase split-gather):
- Core k owns output rows Rk = [1024k, 1024(k+1)). Host pre-transposes
  the slab (AT = A[Rk,:].T, contraction dim j on partitions - no
  on-device PE transposes) and interleaves rows so every SBUF partition
  reads 8KB contiguous per stage DMA regardless of phase width:
  at_ph[s*128+p, r*W+c] = AT[s*(128*R)+r*128+p, lo+c], R j-tiles per
  stage of phase width W.
- Stream in three i-phases: A = cols [0,512), B = [512,768), C =
  [768,1024) (+ values^T/fc in phase B). DVE casts fp32->bf16 into a
  resident 16MB cache ATC [j-part, jt*1024+i]. Row sums d accumulate in
  three PSUM banks, overlapping the stream. All 8 cores stream at the
  device HBM ceiling (~2.3TB/s aggregate), which this layout saturates.
- After each phase: AllGather of that phase's raw d. Gathers 1-2 are
  hidden under the stream; their 48 j-tiles' Y-scales + main matmuls
  also overlap the stream. Only gather-3 (16 j-tiles) is exposed, and
  only 16 tiles' matmuls + epilogue trail it. A warm-up AllGather fed
  straight from a DRAM param fires at t~0 to absorb CC mesh-init and
  launch skew; each gather re-syncs the cores.
- dis distribution: contiguous DMA + sqrt + one PE transpose,
  reciprocal straight from the transpose's PSUM (no 1e-8 guard: d ~
  4096, the shift is ~1.6e-10 relative).
- Y = fc * dis_j in place (bf16); main matmul out^T[o,i] += Y_jt^T @
  ATC_jt over column halves h (h-major; half 0's epilogue overlaps
  half 1); epilogue scales by dis_i via K=1 broadcast matmul; host
  transposes out^T back.
"""
import os
import numpy as np

N, D, OUT = 8192, 128, 128
N_CORES = 8
ROWS = N // N_CORES          # 1024 rows of A per core
NJT = N // 128               # 64 j-tiles
HALF = 512                   # output column half
PH_LO = [0, 512, 768, 896]   # phase column ranges
PH_W = [512, 256, 128, 128]
PH_R = [4, 8, 16, 16]        # j-tiles per stage (8KB per partition row)
PH_NST = [16, 8, 4, 4]       # stages per phase
NPH = 4

_CACHE = {}


def _build():
    import concourse.bacc as bacc
    import concourse.mybir as mybir
    import concourse.tile as tile

    F32, BF16 = mybir.dt.float32, mybir.dt.bfloat16
    nc = bacc.Bacc(None, target_bir_lowering=False, num_devices=N_CORES)

    at_ph = [
        nc.declare_dram_parameter(
            f"at{ph}", [128 * PH_NST[ph], 2048], F32, isOutput=False
        )
        for ph in range(NPH)
    ]
    vt_in = nc.declare_dram_parameter("vt", [D, N], F32, isOutput=False)
    w_in = nc.declare_dram_parameter("w", [D, OUT], F32, isOutput=False)
    bb_in = nc.declare_dram_parameter("bb", [128, OUT], F32, isOutput=False)
    id_in = nc.declare_dram_parameter("ident", [128, 128], F32, isOutput=False)
    outT = nc.declare_dram_parameter("outT", [OUT, ROWS], F32, isOutput=True)

    # gather g covers j-tiles jt with jt%8 in [0,4) / [4,6) / {6} / {7}
    sets = [
        [jt for jt in range(NJT) if jt % 8 < 4],
        [jt for jt in range(NJT) if jt % 8 in (4, 5)],
        [jt for jt in range(NJT) if jt % 8 == 6],
        [jt for jt in range(NJT) if jt % 8 == 7],
    ]

    def col_of(jt):
        k, r = jt // 8, jt % 8
        if r < 4:
            return 4 * k + r
        if r < 6:
            return 32 + 2 * k + (r - 4)
        return (48 if r == 6 else 56) + k

    with tile.TileContext(nc) as tc:
        with (
            tc.tile_pool(name="const", bufs=1) as constp,
            tc.tile_pool(name="stage", bufs=4) as stage,
            tc.tile_pool(name="epi", bufs=2) as epip,
            tc.tile_pool(name="vtb", bufs=1) as vtbp,
            tc.tile_pool(name="small", bufs=1) as small,
            tc.tile_pool(name="ps", bufs=2, space="PSUM") as ps,
            tc.tile_pool(name="psb", bufs=1, space="PSUM") as psb,
            tc.tile_pool(name="po", bufs=1, space="PSUM") as po,
            tc.tile_pool(name="pd", bufs=1, space="PSUM") as pd,
            tc.tile_pool(name="dram", bufs=1, space="DRAM") as dram,
        ):
            # ---- constants ----
            ident = constp.tile([128, 128], F32)
            nc.sync.dma_start(out=ident[:], in_=id_in[:])
            w_sb = constp.tile([D, OUT], F32)
            nc.sync.dma_start(out=w_sb[:], in_=w_in[:])
            w_bf = constp.tile([D, OUT], BF16)
            nc.vector.tensor_copy(w_bf[:], w_sb[:])
            bb_sb = constp.tile([128, OUT], F32)
            nc.sync.dma_start(out=bb_sb[:], in_=bb_in[:])
            ones_bf = constp.tile([128, 1], BF16)
            nc.vector.memset(ones_bf[:], 1.0)
            ones_row = constp.tile([1, 128], F32)
            nc.vector.memset(ones_row[:], 1.0)
            Z = constp.tile([128, 128], F32)
            nc.vector.memset(Z[:], 0.0)

            # warm-up collective: absorbs CC mesh-init + launch skew early,
            # while the stream is DMA-bound and the CC engine is idle.
            wu_loc = dram.tile([8], F32, name="wuloc")
            wu_full = dram.tile([8 * N_CORES], F32, addr_space="Shared", name="wufull")
            nc.sync.dma_start(out=wu_loc[:], in_=ones_row[0:1, 0:8])
            nc.gpsimd.collective_compute(
                "AllGather", mybir.AluOpType.bypass,
                replica_groups=[list(range(N_CORES))],
                ins=[wu_loc[:].opt()], outs=[wu_full[:].opt()],
            )

            # ---- big persistent buffers ----
            ATC = constp.tile([128, NJT * 1024], BF16)   # 16MB transposed A (bf16)
            fcY = constp.tile([128, NJT * 128], BF16)    # 2MB fc_sc, then Y in place
            dis_cols = constp.tile([128, 64], F32)       # dis_j per tile column
            dis_row = constp.tile([1, ROWS], F32)        # local dis_i row

            ATC3 = ATC[:].rearrange("p (j i) -> p j i", j=NJT)

            # phase D reuses phase A's bank (that tile is consumed long
            # before phase D starts; the framework tracks the tile WAR)
            d_tiles = [
                pd.tile([1, PH_W[ph]], F32, tag=f"d{ph}", name=f"dps{ph}")
                for ph in range(3)
            ]
            d_ps = [
                d_tiles[0][:], d_tiles[1][:], d_tiles[2][:],
                d_tiles[0][0:1, 0 : PH_W[3]],
            ]
            oT = [po.tile([128, HALF], F32, tag=f"o{h}", name=f"oT{h}") for h in range(2)]
            dbc = [None, None]
            drows = [None] * NPH
            dfull = [None] * NPH

            # ---- stream phases ----
            for ph in range(NPH):
                if ph == 1:
                    # fc = values @ W + b: streamed here where DMA has slack
                    for c in range(4):
                        vstg = stage.tile([128, 2048], F32, tag="stg")
                        nc.sync.dma_start(
                            out=vstg[:], in_=vt_in[:, c * 2048 : (c + 1) * 2048]
                        )
                        vb = vtbp.tile([128, 2048], BF16, tag="vtb")
                        nc.vector.tensor_copy(vb[:], vstg[:])
                        for m in range(16):
                            nt = c * 16 + m
                            fc_ps = ps.tile([128, OUT], F32, tag="fc")
                            nc.tensor.matmul(
                                fc_ps[:], vb[:, m * 128 : (m + 1) * 128], w_bf[:],
                                start=True, stop=True,
                            )
                            nc.vector.tensor_tensor(
                                out=fcY[:, nt * 128 : (nt + 1) * 128],
                                in0=fc_ps[:], in1=bb_sb[:], op=mybir.AluOpType.add,
                            )
                lo, w, R, nst = PH_LO[ph], PH_W[ph], PH_R[ph], PH_NST[ph]
                for s in range(nst):
                    st = stage.tile([128, 2048], F32, tag="stg")
                    nc.sync.dma_start(
                        out=st[:], in_=at_ph[ph][s * 128 : (s + 1) * 128, :]
                    )
                    nc.vector.tensor_copy(
                        ATC3[:, R * s : R * s + R, lo : lo + w],
                        st[:].rearrange("p (r c) -> p r c", r=R),
                    )
                    for r in range(R):
                        jt = R * s + r
                        nc.tensor.matmul(
                            d_ps[ph], ones_bf[:],
                            ATC[:, jt * 1024 + lo : jt * 1024 + lo + w],
                            start=(s == 0 and r == 0), stop=(s == nst - 1 and r == R - 1),
                        )
                # gather this phase's raw d; the payload DMA reads PSUM
                # directly so the SBUF copy (for local dis_row) is off the
                # critical chain
                drow = small.tile([1, w], F32, tag=f"drow{ph}")
                nc.vector.tensor_copy(drow[:], d_ps[ph])
                dloc = dram.tile([w], F32, name=f"dloc{ph}")
                dfull[ph] = dram.tile(
                    [w * N_CORES], F32, addr_space="Shared", name=f"dfull{ph}"
                )
                nc.sync.dma_start(out=dloc[:], in_=drow[:])
                nc.gpsimd.collective_compute(
                    "AllGather", mybir.AluOpType.bypass,
                    replica_groups=[list(range(N_CORES))],
                    ins=[dloc[:].opt()], outs=[dfull[ph][:].opt()],
                )
                drows[ph] = drow

            # ---- per-gather: distribute dis, scale Y, run main matmuls ----
            ncol = [32, 16, 8, 8]
            cbase = [0, 32, 48, 56]
            for g in range(NPH):
                nc.sync.dma_start(
                    out=Z[0 : ncol[g], :],
                    in_=dfull[g][:].rearrange("(t p) -> t p", p=128),
                )
                nc.scalar.activation(
                    Z[0 : ncol[g], :], Z[0 : ncol[g], :],
                    mybir.ActivationFunctionType.Sqrt,
                )
                zt_ps = ps.tile([128, 128], F32, tag="fc")
                nc.tensor.matmul(zt_ps[:], Z[:], ident[:], is_transpose=True,
                                 start=True, stop=True)
                nc.vector.reciprocal(
                    dis_cols[:, cbase[g] : cbase[g] + ncol[g]], zt_ps[:, 0 : ncol[g]]
                )
                for jt in sets[g]:
                    nc.vector.tensor_scalar(
                        out=fcY[:, jt * 128 : (jt + 1) * 128],
                        in0=fcY[:, jt * 128 : (jt + 1) * 128],
                        scalar1=dis_cols[:, col_of(jt) : col_of(jt) + 1], scalar2=None,
                        op0=mybir.AluOpType.mult,
                    )
                # local dis_row pieces + epilogue broadcasts, as soon as the
                # inputs exist (g0: cols [0,512) -> bc half 0; g2: rest -> bc 1)
                if g == 0:
                    srow = small.tile([1, 512], F32, tag="srow0")
                    nc.scalar.activation(
                        srow[:], drows[0][:], mybir.ActivationFunctionType.Sqrt
                    )
                    nc.vector.reciprocal(dis_row[0:1, 0:512], srow[:])
                    bc_ps = psb.tile([128, HALF], F32, tag="bc")
                    nc.tensor.matmul(
                        bc_ps[:], ones_row[:], dis_row[0:1, 0:512],
                        start=True, stop=True,
                    )
                    dbc[0] = epip.tile([128, HALF], F32, tag="dbc", name="dbc0")
                    nc.vector.tensor_copy(dbc[0][:], bc_ps[:])
                if g == 3:
                    for x in (1, 2, 3):
                        srow = small.tile([1, PH_W[x]], F32, tag=f"srow{x}")
                        nc.scalar.activation(
                            srow[:], drows[x][:], mybir.ActivationFunctionType.Sqrt
                        )
                        nc.vector.reciprocal(
                            dis_row[0:1, PH_LO[x] : PH_LO[x] + PH_W[x]], srow[:]
                        )
                    bc_ps = psb.tile([128, HALF], F32, tag="bc")
                    nc.tensor.matmul(
                        bc_ps[:], ones_row[:], dis_row[0:1, 512:1024],
                        start=True, stop=True,
                    )
                    dbc[1] = epip.tile([128, HALF], F32, tag="dbc", name="dbc1")
                    nc.vector.tensor_copy(dbc[1][:], bc_ps[:])
                for h in range(2):
                    for jt in sets[g]:
                        nc.tensor.matmul(
                            oT[h][:], fcY[:, jt * 128 : (jt + 1) * 128],
                            ATC[:, jt * 1024 + h * HALF : jt * 1024 + (h + 1) * HALF],
                            start=(g == 0 and jt == sets[0][0]),
                            stop=(g == NPH - 1 and jt == sets[NPH - 1][-1]),
                        )
                    if g == NPH - 1:
                        # epilogue for this half overlaps the other half
                        osb = epip.tile([128, HALF], F32, tag="osb")
                        nc.vector.tensor_tensor(
                            out=osb[:], in0=oT[h][:], in1=dbc[h][:],
                            op=mybir.AluOpType.mult,
                        )
                        nc.sync.dma_start(
                            out=outT[:, h * HALF : (h + 1) * HALF], in_=osb[:]
                        )

    nc.compile()
    return nc


def kernel(values, adjacency, W, b):
    from concourse.bass_utils import run_bass_kernel_spmd

    if "nc" not in _CACHE:
        _CACHE["nc"] = _build()
    nc = _CACHE["nc"]

    values = np.asarray(values, dtype=np.float32)
    adjacency = np.asarray(adjacency, dtype=np.float32)
    W = np.asarray(W, dtype=np.float32)
    b = np.asarray(b, dtype=np.float32)

    vt = np.ascontiguousarray(values.T)                  # [D, N]
    bb = np.ascontiguousarray(np.tile(b[None, :], (128, 1)))
    ident = np.eye(128, dtype=np.float32)

    def interleave(block, nst, R, w):
        # block: A rows [nst*R*128/..., hmm] -> see at_ph declaration
        return np.ascontiguousarray(
            block.T.reshape(nst, R, 128, w).transpose(0, 2, 1, 3).reshape(nst * 128, R * w)
        )

    in_maps = []
    for k in range(N_CORES):
        blk = adjacency[k * ROWS : (k + 1) * ROWS]       # [1024, 8192]
        m = {
            "vt": vt, "w": W, "bb": bb, "ident": ident,
        }
        for ph in range(NPH):
            lo, w_, R, nst = PH_LO[ph], PH_W[ph], PH_R[ph], PH_NST[ph]
            m[f"at{ph}"] = interleave(blk[lo : lo + w_, :], nst, R, w_)
        in_maps.append(m)
    trace = bool(int(os.environ.get("GCN_TRACE", "0")))
    res = run_bass_kernel_spmd(nc, in_maps, list(range(N_CORES)), trace=trace)
    if trace and res.exec_time_ns is not None:
        print(f"HW exec time: {res.exec_time_ns} ns")
        _CACHE["exec_time_ns"] = res.exec_time_ns
    out = np.concatenate(
        [res.results[k]["outT"].T for k in range(N_CORES)], axis=0
    ).astype(np.float32)
    return out


# revision 51
# speedup vs baseline: 1.0177x; 1.0177x over previous
"""GCN layer on 8 Trainium2 NeuronCores.

out = D^-1/2 A D^-1/2 (values @ W + b),  A: [8192, 8192] f32 dense.

Strategy (row-parallel, host-interleaved slabs, 3-phase split-gather):
- Core k owns output rows Rk = [1024k, 1024(k+1)). Host pre-transposes
  the slab (AT = A[Rk,:].T, contraction dim j on partitions - no
  on-device PE transposes) and interleaves rows so every SBUF partition
  reads 8KB contiguous per stage DMA regardless of phase width:
  at_ph[s*128+p, r*W+c] = AT[s*(128*R)+r*128+p, lo+c], R j-tiles per
  stage of phase width W.
- Stream in three i-phases: A = cols [0,512), B = [512,768), C =
  [768,1024) (+ values^T/fc in phase B). DVE casts fp32->bf16 into a
  resident 16MB cache ATC [j-part, jt*1024+i]. Row sums d accumulate in
  three PSUM banks, overlapping the stream. All 8 cores stream at the
  device HBM ceiling (~2.3TB/s aggregate), which this layout saturates.
- After each phase: AllGather of that phase's raw d. Gathers 1-2 are
  hidden under the stream; their 48 j-tiles' Y-scales + main matmuls
  also overlap the stream. Only gather-3 (16 j-tiles) is exposed, and
  only 16 tiles' matmuls + epilogue trail it. A warm-up AllGather fed
  straight from a DRAM param fires at t~0 to absorb CC mesh-init and
  launch skew; each gather re-syncs the cores.
- dis distribution: contiguous DMA + sqrt + one PE transpose,
  reciprocal straight from the transpose's PSUM (no 1e-8 guard: d ~
  4096, the shift is ~1.6e-10 relative).
- Y = fc * dis_j in place (bf16); main matmul out^T[o,i] += Y_jt^T @
  ATC_jt over column halves h (h-major; half 0's epilogue overlaps
  half 1); epilogue scales by dis_i via K=1 broadcast matmul; host
  transposes out^T back.
"""
import os
import numpy as np

N, D, OUT = 8192, 128, 128
N_CORES = 8
ROWS = N // N_CORES          # 1024 rows of A per core
NJT = N // 128               # 64 j-tiles
HALF = 512                   # output column half
PH_LO = [0, 512, 768]        # phase column ranges
PH_W = [512, 256, 256]
PH_R = [4, 8, 8]             # j-tiles per stage (8KB per partition row)
PH_NST = [16, 8, 8]          # stages per phase

_CACHE = {}


def _build():
    import concourse.bacc as bacc
    import concourse.mybir as mybir
    import concourse.tile as tile

    F32, BF16 = mybir.dt.float32, mybir.dt.bfloat16
    nc = bacc.Bacc(None, target_bir_lowering=False, num_devices=N_CORES)

    at_ph = [
        nc.declare_dram_parameter(
            f"at{ph}", [128 * PH_NST[ph], 2048], F32, isOutput=False
        )
        for ph in range(3)
    ]
    vt_in = nc.declare_dram_parameter("vt", [D, N], F32, isOutput=False)
    w_in = nc.declare_dram_parameter("w", [D, OUT], F32, isOutput=False)
    bb_in = nc.declare_dram_parameter("bb", [128, OUT], F32, isOutput=False)
    id_in = nc.declare_dram_parameter("ident", [128, 128], F32, isOutput=False)
    outT = nc.declare_dram_parameter("outT", [OUT, ROWS], F32, isOutput=True)

    # gather g covers j-tiles jt with jt%8 in [4,4) / [4,6) / [6,8)
    sets = [
        [jt for jt in range(NJT) if jt % 8 < 4],
        [jt for jt in range(NJT) if jt % 8 in (4, 5)],
        [jt for jt in range(NJT) if jt % 8 >= 6],
    ]

    def col_of(jt):
        k, r = jt // 8, jt % 8
        if r < 4:
            return 4 * k + r
        if r < 6:
            return 32 + 2 * k + (r - 4)
        return 48 + 2 * k + (r - 6)

    with tile.TileContext(nc) as tc:
        with (
            tc.tile_pool(name="const", bufs=1) as constp,
            tc.tile_pool(name="stage", bufs=4) as stage,
            tc.tile_pool(name="epi", bufs=2) as epip,
            tc.tile_pool(name="vtb", bufs=1) as vtbp,
            tc.tile_pool(name="small", bufs=1) as small,
            tc.tile_pool(name="ps", bufs=2, space="PSUM") as ps,
            tc.tile_pool(name="psb", bufs=1, space="PSUM") as psb,
            tc.tile_pool(name="po", bufs=1, space="PSUM") as po,
            tc.tile_pool(name="pd", bufs=1, space="PSUM") as pd,
            tc.tile_pool(name="dram", bufs=1, space="DRAM") as dram,
        ):
            # ---- constants ----
            ident = constp.tile([128, 128], F32)
            nc.sync.dma_start(out=ident[:], in_=id_in[:])
            w_sb = constp.tile([D, OUT], F32)
            nc.sync.dma_start(out=w_sb[:], in_=w_in[:])
            w_bf = constp.tile([D, OUT], BF16)
            nc.vector.tensor_copy(w_bf[:], w_sb[:])
            bb_sb = constp.tile([128, OUT], F32)
            nc.sync.dma_start(out=bb_sb[:], in_=bb_in[:])
            ones_bf = constp.tile([128, 1], BF16)
            nc.vector.memset(ones_bf[:], 1.0)
            ones_row = constp.tile([1, 128], F32)
            nc.vector.memset(ones_row[:], 1.0)
            Z = constp.tile([128, 128], F32)
            nc.vector.memset(Z[:], 0.0)

            # warm-up collective: absorbs CC mesh-init + launch skew early,
            # while the stream is DMA-bound and the CC engine is idle.
            wu_loc = dram.tile([8], F32, name="wuloc")
            wu_full = dram.tile([8 * N_CORES], F32, addr_space="Shared", name="wufull")
            nc.sync.dma_start(out=wu_loc[:], in_=ones_row[0:1, 0:8])
            nc.gpsimd.collective_compute(
                "AllGather", mybir.AluOpType.bypass,
                replica_groups=[list(range(N_CORES))],
                ins=[wu_loc[:].opt()], outs=[wu_full[:].opt()],
            )

            # ---- big persistent buffers ----
            ATC = constp.tile([128, NJT * 1024], BF16)   # 16MB transposed A (bf16)
            fcY = constp.tile([128, NJT * 128], BF16)    # 2MB fc_sc, then Y in place
            dis_cols = constp.tile([128, 64], F32)       # dis_j per tile column
            dis_row = constp.tile([1, ROWS], F32)        # local dis_i row

            ATC3 = ATC[:].rearrange("p (j i) -> p j i", j=NJT)

            d_ps = [
                pd.tile([1, PH_W[ph]], F32, tag=f"d{ph}", name=f"dps{ph}")
                for ph in range(3)
            ]
            oT = [po.tile([128, HALF], F32, tag=f"o{h}", name=f"oT{h}") for h in range(2)]
            dbc = [None, None]
            drows = [None, None, None]
            dfull = [None, None, None]

            # ---- stream phases ----
            for ph in range(3):
                if ph == 1:
                    # fc = values @ W + b: streamed here where DMA has slack
                    for c in range(4):
                        vstg = stage.tile([128, 2048], F32, tag="stg")
                        nc.sync.dma_start(
                            out=vstg[:], in_=vt_in[:, c * 2048 : (c + 1) * 2048]
                        )
                        vb = vtbp.tile([128, 2048], BF16, tag="vtb")
                        nc.vector.tensor_copy(vb[:], vstg[:])
                        for m in range(16):
                            nt = c * 16 + m
                            fc_ps = ps.tile([128, OUT], F32, tag="fc")
                            nc.tensor.matmul(
                                fc_ps[:], vb[:, m * 128 : (m + 1) * 128], w_bf[:],
                                start=True, stop=True,
                            )
                            nc.vector.tensor_tensor(
                                out=fcY[:, nt * 128 : (nt + 1) * 128],
                                in0=fc_ps[:], in1=bb_sb[:], op=mybir.AluOpType.add,
                            )
                lo, w, R, nst = PH_LO[ph], PH_W[ph], PH_R[ph], PH_NST[ph]
                for s in range(nst):
                    st = stage.tile([128, 2048], F32, tag="stg")
                    nc.sync.dma_start(
                        out=st[:], in_=at_ph[ph][s * 128 : (s + 1) * 128, :]
                    )
                    # split the final stage's cast so its d-matmuls (on the
                    # critical chain to the last gather) start earlier
                    nsub = 2 if (ph == 2 and s == nst - 1) else 1
                    for u in range(nsub):
                        r0, r1 = R * u // nsub, R * (u + 1) // nsub
                        nc.vector.tensor_copy(
                            ATC3[:, R * s + r0 : R * s + r1, lo : lo + w],
                            st[:, r0 * w : r1 * w].rearrange(
                                "p (r c) -> p r c", r=r1 - r0
                            ),
                        )
                    for r in range(R):
                        jt = R * s + r
                        nc.tensor.matmul(
                            d_ps[ph][:], ones_bf[:],
                            ATC[:, jt * 1024 + lo : jt * 1024 + lo + w],
                            start=(s == 0 and r == 0), stop=(s == nst - 1 and r == R - 1),
                        )
                # gather this phase's raw d; the payload copy rides the idle
                # Scalar engine, not behind the last casts on Vector
                drow = small.tile([1, w], F32, tag=f"drow{ph}")
                nc.scalar.activation(
                    drow[:], d_ps[ph][:], mybir.ActivationFunctionType.Copy
                )
                dloc = dram.tile([w], F32, name=f"dloc{ph}")
                dfull[ph] = dram.tile(
                    [w * N_CORES], F32, addr_space="Shared", name=f"dfull{ph}"
                )
                nc.sync.dma_start(out=dloc[:], in_=drow[:])
                nc.gpsimd.collective_compute(
                    "AllGather", mybir.AluOpType.bypass,
                    replica_groups=[list(range(N_CORES))],
                    ins=[dloc[:].opt()], outs=[dfull[ph][:].opt()],
                )
                drows[ph] = drow

            # ---- per-gather: distribute dis, scale Y, run main matmuls ----
            ncol = [32, 16, 16]
            cbase = [0, 32, 48]
            for g in range(3):
                nc.sync.dma_start(
                    out=Z[0 : ncol[g], :],
                    in_=dfull[g][:].rearrange("(t p) -> t p", p=128),
                )
                nc.scalar.activation(
                    Z[0 : ncol[g], :], Z[0 : ncol[g], :],
                    mybir.ActivationFunctionType.Sqrt,
                )
                zt_ps = ps.tile([128, 128], F32, tag="fc")
                nc.tensor.matmul(zt_ps[:], Z[:], ident[:], is_transpose=True,
                                 start=True, stop=True)
                nc.vector.reciprocal(
                    dis_cols[:, cbase[g] : cbase[g] + ncol[g]], zt_ps[:, 0 : ncol[g]]
                )
                for jt in sets[g]:
                    nc.vector.tensor_scalar(
                        out=fcY[:, jt * 128 : (jt + 1) * 128],
                        in0=fcY[:, jt * 128 : (jt + 1) * 128],
                        scalar1=dis_cols[:, col_of(jt) : col_of(jt) + 1], scalar2=None,
                        op0=mybir.AluOpType.mult,
                    )
                # local dis_row pieces + epilogue broadcasts, as soon as the
                # inputs exist (g0: cols [0,512) -> bc half 0; g2: rest -> bc 1)
                if g == 0:
                    srow = small.tile([1, 512], F32, tag="srow0")
                    nc.scalar.activation(
                        srow[:], drows[0][:], mybir.ActivationFunctionType.Sqrt
                    )
                    nc.vector.reciprocal(dis_row[0:1, 0:512], srow[:])
                    bc_ps = psb.tile([128, HALF], F32, tag="bc")
                    nc.tensor.matmul(
                        bc_ps[:], ones_row[:], dis_row[0:1, 0:512],
                        start=True, stop=True,
                    )
                    dbc[0] = epip.tile([128, HALF], F32, tag="dbc", name="dbc0")
                    nc.vector.tensor_copy(dbc[0][:], bc_ps[:])
                if g == 2:
                    for x in (1, 2):
                        srow = small.tile([1, 256], F32, tag=f"srow{x}")
                        nc.scalar.activation(
                            srow[:], drows[x][:], mybir.ActivationFunctionType.Sqrt
                        )
                        nc.vector.reciprocal(
                            dis_row[0:1, 256 + 256 * x : 512 + 256 * x], srow[:]
                        )
                    bc_ps = psb.tile([128, HALF], F32, tag="bc")
                    nc.tensor.matmul(
                        bc_ps[:], ones_row[:], dis_row[0:1, 512:1024],
                        start=True, stop=True,
                    )
                    dbc[1] = epip.tile([128, HALF], F32, tag="dbc", name="dbc1")
                    nc.vector.tensor_copy(dbc[1][:], bc_ps[:])
                for h in range(2):
                    for jt in sets[g]:
                        nc.tensor.matmul(
                            oT[h][:], fcY[:, jt * 128 : (jt + 1) * 128],
                            ATC[:, jt * 1024 + h * HALF : jt * 1024 + (h + 1) * HALF],
                            start=(g == 0 and jt == sets[0][0]),
                            stop=(g == 2 and jt == sets[2][-1]),
                        )
                    if g == 2:
                        # epilogue for this half overlaps the other half
                        osb = epip.tile([128, HALF], F32, tag="osb")
                        nc.vector.tensor_tensor(
                            out=osb[:], in0=oT[h][:], in1=dbc[h][:],
                            op=mybir.AluOpType.mult,
                        )
                        nc.sync.dma_start(
                            out=outT[:, h * HALF : (h + 1) * HALF], in_=osb[:]
                        )

    nc.compile()
    return nc


def kernel(values, adjacency, W, b):
    from concourse.bass_utils import run_bass_kernel_spmd

    if "nc" not in _CACHE:
        _CACHE["nc"] = _build()
    nc = _CACHE["nc"]

    values = np.asarray(values, dtype=np.float32)
    adjacency = np.asarray(adjacency, dtype=np.float32)
    W = np.asarray(W, dtype=np.float32)
    b = np.asarray(b, dtype=np.float32)

    vt = np.ascontiguousarray(values.T)                  # [D, N]
    bb = np.ascontiguousarray(np.tile(b[None, :], (128, 1)))
    ident = np.eye(128, dtype=np.float32)

    def interleave(block, nst, R, w):
        # block: A rows [nst*R*128/..., hmm] -> see at_ph declaration
        return np.ascontiguousarray(
            block.T.reshape(nst, R, 128, w).transpose(0, 2, 1, 3).reshape(nst * 128, R * w)
        )

    in_maps = []
    for k in range(N_CORES):
        blk = adjacency[k * ROWS : (k + 1) * ROWS]       # [1024, 8192]
        m = {
            "vt": vt, "w": W, "bb": bb, "ident": ident,
        }
        for ph in range(3):
            lo, w_, R, nst = PH_LO[ph], PH_W[ph], PH_R[ph], PH_NST[ph]
            m[f"at{ph}"] = interleave(blk[lo : lo + w_, :], nst, R, w_)
        in_maps.append(m)
    trace = bool(int(os.environ.get("GCN_TRACE", "0")))
    res = run_bass_kernel_spmd(nc, in_maps, list(range(N_CORES)), trace=trace)
    if trace and res.exec_time_ns is not None:
        print(f"HW exec time: {res.exec_time_ns} ns")
        _CACHE["exec_time_ns"] = res.exec_time_ns
    out = np.concatenate(
        [res.results[k]["outT"].T for k in range(N_CORES)], axis=0
    ).astype(np.float32)
    return out


# revision 52
# speedup vs baseline: 1.0542x; 1.0359x over previous
"""GCN layer on 8 Trainium2 NeuronCores.

out = D^-1/2 A D^-1/2 (values @ W + b),  A: [8192, 8192] f32 dense.

Strategy (row-parallel, host-interleaved slabs, 3-phase split-gather):
- Core k owns output rows Rk = [1024k, 1024(k+1)). Host pre-transposes
  the slab (AT = A[Rk,:].T, contraction dim j on partitions - no
  on-device PE transposes) and interleaves rows so every SBUF partition
  reads 8KB contiguous per stage DMA regardless of phase width:
  at_ph[s*128+p, r*W+c] = AT[s*(128*R)+r*128+p, lo+c], R j-tiles per
  stage of phase width W.
- Stream in three i-phases: A = cols [0,512), B = [512,768), C =
  [768,1024) (+ values^T/fc in phase B). DVE casts fp32->bf16 into a
  resident 16MB cache ATC [j-part, jt*1024+i]. Row sums d accumulate in
  three PSUM banks, overlapping the stream. All 8 cores stream at the
  device HBM ceiling (~2.3TB/s aggregate), which this layout saturates.
- After each phase: AllGather of that phase's raw d. Gathers 1-2 are
  hidden under the stream; their 48 j-tiles' Y-scales + main matmuls
  also overlap the stream. Only gather-3 (16 j-tiles) is exposed, and
  only 16 tiles' matmuls + epilogue trail it. A warm-up AllGather fed
  straight from a DRAM param fires at t~0 to absorb CC mesh-init and
  launch skew; each gather re-syncs the cores.
- dis distribution: contiguous DMA + sqrt + one PE transpose,
  reciprocal straight from the transpose's PSUM (no 1e-8 guard: d ~
  4096, the shift is ~1.6e-10 relative).
- Y = fc * dis_j in place (bf16); main matmul out^T[o,i] += Y_jt^T @
  ATC_jt over column halves h (h-major; half 0's epilogue overlaps
  half 1); epilogue scales by dis_i via K=1 broadcast matmul; host
  transposes out^T back.
"""
import os
import numpy as np

N, D, OUT = 8192, 128, 128
N_CORES = 8
ROWS = N // N_CORES          # 1024 rows of A per core
NJT = N // 128               # 64 j-tiles
HALF = 512                   # output column half
PH_LO = [0, 512, 768]        # phase column ranges
PH_W = [512, 256, 256]
PH_R = [4, 8, 8]             # j-tiles per stage (8KB per partition row)
PH_NST = [16, 8, 8]          # stages per phase

_CACHE = {}


def _build():
    import concourse.bacc as bacc
    import concourse.mybir as mybir
    import concourse.tile as tile

    F32, BF16 = mybir.dt.float32, mybir.dt.bfloat16
    nc = bacc.Bacc(None, target_bir_lowering=False, num_devices=N_CORES)

    at_ph = [
        nc.declare_dram_parameter(
            f"at{ph}", [128 * PH_NST[ph], 2048], F32, isOutput=False
        )
        for ph in range(3)
    ]
    vt_in = nc.declare_dram_parameter("vt", [D, N], F32, isOutput=False)
    w_in = nc.declare_dram_parameter("w", [D, OUT], F32, isOutput=False)
    bb_in = nc.declare_dram_parameter("bb", [128, OUT], F32, isOutput=False)
    id_in = nc.declare_dram_parameter("ident", [128, 128], F32, isOutput=False)
    outT = nc.declare_dram_parameter("outT", [OUT, ROWS], F32, isOutput=True)

    # gather g covers j-tiles jt with jt%8 in [4,4) / [4,6) / [6,8)
    sets = [
        [jt for jt in range(NJT) if jt % 8 < 4],
        [jt for jt in range(NJT) if jt % 8 in (4, 5)],
        [jt for jt in range(NJT) if jt % 8 >= 6],
    ]

    def col_of(jt):
        k, r = jt // 8, jt % 8
        if r < 4:
            return 4 * k + r
        if r < 6:
            return 32 + 2 * k + (r - 4)
        return 48 + 2 * k + (r - 6)

    with tile.TileContext(nc) as tc:
        with (
            tc.tile_pool(name="const", bufs=1) as constp,
            tc.tile_pool(name="stage", bufs=4) as stage,
            tc.tile_pool(name="epi", bufs=2) as epip,
            tc.tile_pool(name="vtb", bufs=1) as vtbp,
            tc.tile_pool(name="small", bufs=1) as small,
            tc.tile_pool(name="ps", bufs=2, space="PSUM") as ps,
            tc.tile_pool(name="psb", bufs=1, space="PSUM") as psb,
            tc.tile_pool(name="po", bufs=1, space="PSUM") as po,
            tc.tile_pool(name="pd", bufs=1, space="PSUM") as pd,
            tc.tile_pool(name="dram", bufs=1, space="DRAM") as dram,
        ):
            # ---- constants ----
            ident = constp.tile([128, 128], F32)
            nc.sync.dma_start(out=ident[:], in_=id_in[:])
            w_sb = constp.tile([D, OUT], F32)
            nc.sync.dma_start(out=w_sb[:], in_=w_in[:])
            w_bf = constp.tile([D, OUT], BF16)
            nc.vector.tensor_copy(w_bf[:], w_sb[:])
            bb_sb = constp.tile([128, OUT], F32)
            nc.sync.dma_start(out=bb_sb[:], in_=bb_in[:])
            ones_bf = constp.tile([128, 1], BF16)
            nc.vector.memset(ones_bf[:], 1.0)
            ones_row = constp.tile([1, 128], F32)
            nc.vector.memset(ones_row[:], 1.0)
            Z = constp.tile([128, 128], F32)
            nc.vector.memset(Z[:], 0.0)

            # warm-up collective: absorbs CC mesh-init + launch skew early,
            # while the stream is DMA-bound and the CC engine is idle.
            wu_loc = dram.tile([8], F32, name="wuloc")
            wu_full = dram.tile([8 * N_CORES], F32, addr_space="Shared", name="wufull")
            nc.sync.dma_start(out=wu_loc[:], in_=ones_row[0:1, 0:8])
            nc.gpsimd.collective_compute(
                "AllGather", mybir.AluOpType.bypass,
                replica_groups=[list(range(N_CORES))],
                ins=[wu_loc[:].opt()], outs=[wu_full[:].opt()],
            )

            # ---- big persistent buffers ----
            ATC = constp.tile([128, NJT * 1024], BF16)   # 16MB transposed A (bf16)
            fcY = constp.tile([128, NJT * 128], BF16)    # 2MB fc_sc, then Y in place
            dis_cols = constp.tile([128, 64], F32)       # dis_j per tile column
            dis_row = constp.tile([1, ROWS], F32)        # local dis_i row

            ATC3 = ATC[:].rearrange("p (j i) -> p j i", j=NJT)

            d_ps = [
                pd.tile([1, PH_W[ph]], F32, tag=f"d{ph}", name=f"dps{ph}")
                for ph in range(3)
            ]
            oT = [po.tile([128, HALF], F32, tag=f"o{h}", name=f"oT{h}") for h in range(2)]
            dbc = [None, None]
            drows = [None, None, None]
            dfull = [None, None, None]

            # ---- stream phases ----
            for ph in range(3):
                if ph == 1:
                    # fc = values @ W + b: streamed here where DMA has slack
                    for c in range(4):
                        vstg = stage.tile([128, 2048], F32, tag="stg")
                        nc.sync.dma_start(
                            out=vstg[:], in_=vt_in[:, c * 2048 : (c + 1) * 2048]
                        )
                        vb = vtbp.tile([128, 2048], BF16, tag="vtb")
                        nc.vector.tensor_copy(vb[:], vstg[:])
                        for m in range(16):
                            nt = c * 16 + m
                            fc_ps = ps.tile([128, OUT], F32, tag="fc")
                            nc.tensor.matmul(
                                fc_ps[:], vb[:, m * 128 : (m + 1) * 128], w_bf[:],
                                start=True, stop=True,
                            )
                            nc.vector.tensor_tensor(
                                out=fcY[:, nt * 128 : (nt + 1) * 128],
                                in0=fc_ps[:], in1=bb_sb[:], op=mybir.AluOpType.add,
                            )
                lo, w, R, nst = PH_LO[ph], PH_W[ph], PH_R[ph], PH_NST[ph]
                for s in range(nst):
                    # the final stage (critical chain to the last gather) is
                    # split into two DMA/cast/d-matmul pipelines so the d sum
                    # completes sooner after the last byte lands
                    nsub = 2 if (ph == 2 and s == nst - 1) else 1
                    for u in range(nsub):
                        r0, r1 = R * u // nsub, R * (u + 1) // nsub
                        st = stage.tile([128, 2048], F32, tag="stg")
                        nc.sync.dma_start(
                            out=st[:, 0 : (r1 - r0) * w],
                            in_=at_ph[ph][s * 128 : (s + 1) * 128, r0 * w : r1 * w],
                        )
                        nc.vector.tensor_copy(
                            ATC3[:, R * s + r0 : R * s + r1, lo : lo + w],
                            st[:, 0 : (r1 - r0) * w].rearrange(
                                "p (r c) -> p r c", r=r1 - r0
                            ),
                        )
                        for r in range(r0, r1):
                            jt = R * s + r
                            nc.tensor.matmul(
                                d_ps[ph][:], ones_bf[:],
                                ATC[:, jt * 1024 + lo : jt * 1024 + lo + w],
                                start=(s == 0 and r == 0),
                                stop=(s == nst - 1 and r == R - 1),
                            )
                # gather this phase's raw d; the payload copy rides the idle
                # Scalar engine, not behind the last casts on Vector
                drow = small.tile([1, w], F32, tag=f"drow{ph}")
                nc.scalar.activation(
                    drow[:], d_ps[ph][:], mybir.ActivationFunctionType.Copy
                )
                dloc = dram.tile([w], F32, name=f"dloc{ph}")
                dfull[ph] = dram.tile(
                    [w * N_CORES], F32, addr_space="Shared", name=f"dfull{ph}"
                )
                nc.sync.dma_start(out=dloc[:], in_=drow[:])
                nc.gpsimd.collective_compute(
                    "AllGather", mybir.AluOpType.bypass,
                    replica_groups=[list(range(N_CORES))],
                    ins=[dloc[:].opt()], outs=[dfull[ph][:].opt()],
                )
                drows[ph] = drow

            # ---- per-gather: distribute dis, scale Y, run main matmuls ----
            ncol = [32, 16, 16]
            cbase = [0, 32, 48]
            for g in range(3):
                nc.sync.dma_start(
                    out=Z[0 : ncol[g], :],
                    in_=dfull[g][:].rearrange("(t p) -> t p", p=128),
                )
                nc.scalar.activation(
                    Z[0 : ncol[g], :], Z[0 : ncol[g], :],
                    mybir.ActivationFunctionType.Sqrt,
                )
                zt_ps = ps.tile([128, 128], F32, tag="fc")
                nc.tensor.matmul(zt_ps[:], Z[:], ident[:], is_transpose=True,
                                 start=True, stop=True)
                nc.vector.reciprocal(
                    dis_cols[:, cbase[g] : cbase[g] + ncol[g]], zt_ps[:, 0 : ncol[g]]
                )
                for jt in sets[g]:
                    nc.vector.tensor_scalar(
                        out=fcY[:, jt * 128 : (jt + 1) * 128],
                        in0=fcY[:, jt * 128 : (jt + 1) * 128],
                        scalar1=dis_cols[:, col_of(jt) : col_of(jt) + 1], scalar2=None,
                        op0=mybir.AluOpType.mult,
                    )
                # local dis_row pieces + epilogue broadcasts, as soon as the
                # inputs exist (g0: cols [0,512) -> bc half 0; g2: rest -> bc 1)
                if g == 0:
                    srow = small.tile([1, 512], F32, tag="srow0")
                    nc.scalar.activation(
                        srow[:], drows[0][:], mybir.ActivationFunctionType.Sqrt
                    )
                    nc.vector.reciprocal(dis_row[0:1, 0:512], srow[:])
                    bc_ps = psb.tile([128, HALF], F32, tag="bc")
                    nc.tensor.matmul(
                        bc_ps[:], ones_row[:], dis_row[0:1, 0:512],
                        start=True, stop=True,
                    )
                    dbc[0] = epip.tile([128, HALF], F32, tag="dbc", name="dbc0")
                    nc.vector.tensor_copy(dbc[0][:], bc_ps[:])
                if g == 2:
                    for x in (1, 2):
                        srow = small.tile([1, 256], F32, tag=f"srow{x}")
                        nc.scalar.activation(
                            srow[:], drows[x][:], mybir.ActivationFunctionType.Sqrt
                        )
                        nc.vector.reciprocal(
                            dis_row[0:1, 256 + 256 * x : 512 + 256 * x], srow[:]
                        )
                    bc_ps = psb.tile([128, HALF], F32, tag="bc")
                    nc.tensor.matmul(
                        bc_ps[:], ones_row[:], dis_row[0:1, 512:1024],
                        start=True, stop=True,
                    )
                    dbc[1] = epip.tile([128, HALF], F32, tag="dbc", name="dbc1")
                    nc.vector.tensor_copy(dbc[1][:], bc_ps[:])
                for h in range(2):
                    for jt in sets[g]:
                        nc.tensor.matmul(
                            oT[h][:], fcY[:, jt * 128 : (jt + 1) * 128],
                            ATC[:, jt * 1024 + h * HALF : jt * 1024 + (h + 1) * HALF],
                            start=(g == 0 and jt == sets[0][0]),
                            stop=(g == 2 and jt == sets[2][-1]),
                        )
                    if g == 2:
                        # epilogue for this half overlaps the other half
                        osb = epip.tile([128, HALF], F32, tag="osb")
                        nc.vector.tensor_tensor(
                            out=osb[:], in0=oT[h][:], in1=dbc[h][:],
                            op=mybir.AluOpType.mult,
                        )
                        nc.sync.dma_start(
                            out=outT[:, h * HALF : (h + 1) * HALF], in_=osb[:]
                        )

    nc.compile()
    return nc


def kernel(values, adjacency, W, b):
    from concourse.bass_utils import run_bass_kernel_spmd

    if "nc" not in _CACHE:
        _CACHE["nc"] = _build()
    nc = _CACHE["nc"]

    values = np.asarray(values, dtype=np.float32)
    adjacency = np.asarray(adjacency, dtype=np.float32)
    W = np.asarray(W, dtype=np.float32)
    b = np.asarray(b, dtype=np.float32)

    vt = np.ascontiguousarray(values.T)                  # [D, N]
    bb = np.ascontiguousarray(np.tile(b[None, :], (128, 1)))
    ident = np.eye(128, dtype=np.float32)

    def interleave(block, nst, R, w):
        # block: A rows [nst*R*128/..., hmm] -> see at_ph declaration
        return np.ascontiguousarray(
            block.T.reshape(nst, R, 128, w).transpose(0, 2, 1, 3).reshape(nst * 128, R * w)
        )

    in_maps = []
    for k in range(N_CORES):
        blk = adjacency[k * ROWS : (k + 1) * ROWS]       # [1024, 8192]
        m = {
            "vt": vt, "w": W, "bb": bb, "ident": ident,
        }
        for ph in range(3):
            lo, w_, R, nst = PH_LO[ph], PH_W[ph], PH_R[ph], PH_NST[ph]
            m[f"at{ph}"] = interleave(blk[lo : lo + w_, :], nst, R, w_)
        in_maps.append(m)
    trace = bool(int(os.environ.get("GCN_TRACE", "0")))
    res = run_bass_kernel_spmd(nc, in_maps, list(range(N_CORES)), trace=trace)
    if trace and res.exec_time_ns is not None:
        print(f"HW exec time: {res.exec_time_ns} ns")
        _CACHE["exec_time_ns"] = res.exec_time_ns
    out = np.concatenate(
        [res.results[k]["outT"].T for k in range(N_CORES)], axis=0
    ).astype(np.float32)
    return out


# revision 53
# speedup vs baseline: 1.0763x; 1.0210x over previous
"""GCN layer on 8 Trainium2 NeuronCores.

out = D^-1/2 A D^-1/2 (values @ W + b),  A: [8192, 8192] f32 dense.

Strategy (row-parallel, host-interleaved slabs, 3-phase split-gather):
- Core k owns output rows Rk = [1024k, 1024(k+1)). Host pre-transposes
  the slab (AT = A[Rk,:].T, contraction dim j on partitions - no
  on-device PE transposes) and interleaves rows so every SBUF partition
  reads 8KB contiguous per stage DMA regardless of phase width:
  at_ph[s*128+p, r*W+c] = AT[s*(128*R)+r*128+p, lo+c], R j-tiles per
  stage of phase width W.
- Stream in three i-phases: A = cols [0,512), B = [512,768), C =
  [768,1024) (+ values^T/fc in phase B). DVE casts fp32->bf16 into a
  resident 16MB cache ATC [j-part, jt*1024+i]. Row sums d accumulate in
  three PSUM banks, overlapping the stream. All 8 cores stream at the
  device HBM ceiling (~2.3TB/s aggregate), which this layout saturates.
- After each phase: AllGather of that phase's raw d. Gathers 1-2 are
  hidden under the stream; their 48 j-tiles' Y-scales + main matmuls
  also overlap the stream. Only gather-3 (16 j-tiles) is exposed, and
  only 16 tiles' matmuls + epilogue trail it. A warm-up AllGather fed
  straight from a DRAM param fires at t~0 to absorb CC mesh-init and
  launch skew; each gather re-syncs the cores.
- dis distribution: contiguous DMA + sqrt + one PE transpose,
  reciprocal straight from the transpose's PSUM (no 1e-8 guard: d ~
  4096, the shift is ~1.6e-10 relative).
- Y = fc * dis_j in place (bf16); main matmul out^T[o,i] += Y_jt^T @
  ATC_jt over column halves h (h-major; half 0's epilogue overlaps
  half 1); epilogue scales by dis_i via K=1 broadcast matmul; host
  transposes out^T back.
"""
import os
import numpy as np

N, D, OUT = 8192, 128, 128
N_CORES = 8
ROWS = N // N_CORES          # 1024 rows of A per core
NJT = N // 128               # 64 j-tiles
HALF = 512                   # output column half
PH_LO = [0, 512, 768]        # phase column ranges
PH_W = [512, 256, 256]
PH_R = [4, 8, 8]             # j-tiles per stage (8KB per partition row)
PH_NST = [16, 8, 8]          # stages per phase

_CACHE = {}


def _build():
    import concourse.bacc as bacc
    import concourse.mybir as mybir
    import concourse.tile as tile

    F32, BF16 = mybir.dt.float32, mybir.dt.bfloat16
    nc = bacc.Bacc(None, target_bir_lowering=False, num_devices=N_CORES)

    at_ph = [
        nc.declare_dram_parameter(
            f"at{ph}", [128 * PH_NST[ph], 2048], F32, isOutput=False
        )
        for ph in range(3)
    ]
    vt_in = nc.declare_dram_parameter("vt", [D, N], F32, isOutput=False)
    w_in = nc.declare_dram_parameter("w", [D, OUT], F32, isOutput=False)
    bb_in = nc.declare_dram_parameter("bb", [128, OUT], F32, isOutput=False)
    id_in = nc.declare_dram_parameter("ident", [128, 128], F32, isOutput=False)
    outT = nc.declare_dram_parameter("outT", [OUT, ROWS], F32, isOutput=True)

    # gather g covers j-tiles jt with jt%8 in [4,4) / [4,6) / [6,8)
    sets = [
        [jt for jt in range(NJT) if jt % 8 < 4],
        [jt for jt in range(NJT) if jt % 8 in (4, 5)],
        [jt for jt in range(NJT) if jt % 8 >= 6],
    ]

    def col_of(jt):
        k, r = jt // 8, jt % 8
        if r < 4:
            return 4 * k + r
        if r < 6:
            return 32 + 2 * k + (r - 4)
        return 48 + 2 * k + (r - 6)

    with tile.TileContext(nc) as tc:
        with (
            tc.tile_pool(name="const", bufs=1) as constp,
            tc.tile_pool(name="stage", bufs=4) as stage,
            tc.tile_pool(name="epi", bufs=2) as epip,
            tc.tile_pool(name="vtb", bufs=1) as vtbp,
            tc.tile_pool(name="small", bufs=1) as small,
            tc.tile_pool(name="ps", bufs=2, space="PSUM") as ps,
            tc.tile_pool(name="psb", bufs=1, space="PSUM") as psb,
            tc.tile_pool(name="po", bufs=1, space="PSUM") as po,
            tc.tile_pool(name="pd", bufs=1, space="PSUM") as pd,
            tc.tile_pool(name="dram", bufs=1, space="DRAM") as dram,
        ):
            # ---- constants ----
            ident = constp.tile([128, 128], F32)
            nc.sync.dma_start(out=ident[:], in_=id_in[:])
            w_sb = constp.tile([D, OUT], F32)
            nc.sync.dma_start(out=w_sb[:], in_=w_in[:])
            w_bf = constp.tile([D, OUT], BF16)
            nc.vector.tensor_copy(w_bf[:], w_sb[:])
            bb_sb = constp.tile([128, OUT], F32)
            nc.sync.dma_start(out=bb_sb[:], in_=bb_in[:])
            ones_bf = constp.tile([128, 1], BF16)
            nc.vector.memset(ones_bf[:], 1.0)
            ones_row = constp.tile([1, 128], F32)
            nc.vector.memset(ones_row[:], 1.0)
            Z = constp.tile([128, 128], F32)
            nc.vector.memset(Z[:], 0.0)

            # warm-up collective: absorbs CC mesh-init + launch skew early,
            # while the stream is DMA-bound and the CC engine is idle.
            wu_loc = dram.tile([8], F32, name="wuloc")
            wu_full = dram.tile([8 * N_CORES], F32, addr_space="Shared", name="wufull")
            nc.sync.dma_start(out=wu_loc[:], in_=ones_row[0:1, 0:8])
            nc.gpsimd.collective_compute(
                "AllGather", mybir.AluOpType.bypass,
                replica_groups=[list(range(N_CORES))],
                ins=[wu_loc[:].opt()], outs=[wu_full[:].opt()],
            )

            # ---- big persistent buffers ----
            ATC = constp.tile([128, NJT * 1024], BF16)   # 16MB transposed A (bf16)
            fcY = constp.tile([128, NJT * 128], BF16)    # 2MB fc_sc, then Y in place
            dis_cols = constp.tile([128, 64], F32)       # dis_j per tile column
            dis_row = constp.tile([1, ROWS], F32)        # local dis_i row

            ATC3 = ATC[:].rearrange("p (j i) -> p j i", j=NJT)

            d_ps = [
                pd.tile([1, PH_W[ph]], F32, tag=f"d{ph}", name=f"dps{ph}")
                for ph in range(3)
            ]
            oT = [po.tile([128, HALF], F32, tag=f"o{h}", name=f"oT{h}") for h in range(2)]
            dbc = [None, None]
            drows = [None, None, None]
            dfull = [None, None, None]

            # ---- stream phases ----
            for ph in range(3):
                if ph == 1:
                    # fc = values @ W + b: streamed here where DMA has slack
                    for c in range(4):
                        vstg = stage.tile([128, 2048], F32, tag="stg")
                        nc.sync.dma_start(
                            out=vstg[:], in_=vt_in[:, c * 2048 : (c + 1) * 2048]
                        )
                        vb = vtbp.tile([128, 2048], BF16, tag="vtb")
                        nc.vector.tensor_copy(vb[:], vstg[:])
                        for m in range(16):
                            nt = c * 16 + m
                            fc_ps = ps.tile([128, OUT], F32, tag="fc")
                            nc.tensor.matmul(
                                fc_ps[:], vb[:, m * 128 : (m + 1) * 128], w_bf[:],
                                start=True, stop=True,
                            )
                            nc.vector.tensor_tensor(
                                out=fcY[:, nt * 128 : (nt + 1) * 128],
                                in0=fc_ps[:], in1=bb_sb[:], op=mybir.AluOpType.add,
                            )
                lo, w, R, nst = PH_LO[ph], PH_W[ph], PH_R[ph], PH_NST[ph]
                for s in range(nst):
                    # the final stage (critical chain to the last gather) is
                    # split into two DMA/cast/d-matmul pipelines so the d sum
                    # completes sooner after the last byte lands
                    nsub = 2 if (ph == 2 and s == nst - 1) else 1
                    for u in range(nsub):
                        r0, r1 = R * u // nsub, R * (u + 1) // nsub
                        st = stage.tile([128, 2048], F32, tag="stg")
                        nc.sync.dma_start(
                            out=st[:, 0 : (r1 - r0) * w],
                            in_=at_ph[ph][s * 128 : (s + 1) * 128, r0 * w : r1 * w],
                        )
                        nc.vector.tensor_copy(
                            ATC3[:, R * s + r0 : R * s + r1, lo : lo + w],
                            st[:, 0 : (r1 - r0) * w].rearrange(
                                "p (r c) -> p r c", r=r1 - r0
                            ),
                        )
                        for r in range(r0, r1):
                            jt = R * s + r
                            nc.tensor.matmul(
                                d_ps[ph][:], ones_bf[:],
                                ATC[:, jt * 1024 + lo : jt * 1024 + lo + w],
                                start=(s == 0 and r == 0),
                                stop=(s == nst - 1 and r == R - 1),
                            )
                # gather this phase's raw d; the payload copy rides the idle
                # Scalar engine, not behind the last casts on Vector
                drow = small.tile([1, w], F32, tag=f"drow{ph}")
                nc.scalar.activation(
                    drow[:], d_ps[ph][:], mybir.ActivationFunctionType.Copy
                )
                dloc = dram.tile([w], F32, name=f"dloc{ph}")
                dfull[ph] = dram.tile(
                    [w * N_CORES], F32, addr_space="Shared", name=f"dfull{ph}"
                )
                nc.sync.dma_start(out=dloc[:], in_=drow[:])
                nc.gpsimd.collective_compute(
                    "AllGather", mybir.AluOpType.bypass,
                    replica_groups=[list(range(N_CORES))],
                    ins=[dloc[:].opt()], outs=[dfull[ph][:].opt()],
                )
                drows[ph] = drow

            # ---- per-gather: distribute dis, scale Y, run main matmuls ----
            ncol = [32, 16, 16]
            cbase = [0, 32, 48]
            for g in range(3):
                # local dis_row pieces + epilogue broadcasts FIRST: their
                # inputs are ready (drow of earlier phases), so they fill the
                # gather-wait window instead of delaying the final epilogue
                if g == 0:
                    srow = small.tile([1, 512], F32, tag="srow0")
                    nc.scalar.activation(
                        srow[:], drows[0][:], mybir.ActivationFunctionType.Sqrt
                    )
                    nc.vector.reciprocal(dis_row[0:1, 0:512], srow[:])
                    bc_ps = psb.tile([128, HALF], F32, tag="bc")
                    nc.tensor.matmul(
                        bc_ps[:], ones_row[:], dis_row[0:1, 0:512],
                        start=True, stop=True,
                    )
                    dbc[0] = epip.tile([128, HALF], F32, tag="dbc", name="dbc0")
                    nc.vector.tensor_copy(dbc[0][:], bc_ps[:])
                if g == 2:
                    for x in (1, 2):
                        srow = small.tile([1, 256], F32, tag=f"srow{x}")
                        nc.scalar.activation(
                            srow[:], drows[x][:], mybir.ActivationFunctionType.Sqrt
                        )
                        nc.vector.reciprocal(
                            dis_row[0:1, 256 + 256 * x : 512 + 256 * x], srow[:]
                        )
                    bc_ps = psb.tile([128, HALF], F32, tag="bc")
                    nc.tensor.matmul(
                        bc_ps[:], ones_row[:], dis_row[0:1, 512:1024],
                        start=True, stop=True,
                    )
                    dbc[1] = epip.tile([128, HALF], F32, tag="dbc", name="dbc1")
                    nc.vector.tensor_copy(dbc[1][:], bc_ps[:])
                nc.sync.dma_start(
                    out=Z[0 : ncol[g], :],
                    in_=dfull[g][:].rearrange("(t p) -> t p", p=128),
                )
                nc.scalar.activation(
                    Z[0 : ncol[g], :], Z[0 : ncol[g], :],
                    mybir.ActivationFunctionType.Sqrt,
                )
                zt_ps = ps.tile([128, 128], F32, tag="fc")
                nc.tensor.matmul(zt_ps[:], Z[:], ident[:], is_transpose=True,
                                 start=True, stop=True)
                nc.vector.reciprocal(
                    dis_cols[:, cbase[g] : cbase[g] + ncol[g]], zt_ps[:, 0 : ncol[g]]
                )
                # Y scales: split across Vector and Scalar so the tile-by-tile
                # pacing of the main matmuls is twice as fast
                for x, jt in enumerate(sets[g]):
                    if x % 2 == 0:
                        nc.vector.tensor_scalar(
                            out=fcY[:, jt * 128 : (jt + 1) * 128],
                            in0=fcY[:, jt * 128 : (jt + 1) * 128],
                            scalar1=dis_cols[:, col_of(jt) : col_of(jt) + 1],
                            scalar2=None, op0=mybir.AluOpType.mult,
                        )
                    else:
                        nc.scalar.activation(
                            fcY[:, jt * 128 : (jt + 1) * 128],
                            fcY[:, jt * 128 : (jt + 1) * 128],
                            mybir.ActivationFunctionType.Copy,
                            scale=dis_cols[:, col_of(jt) : col_of(jt) + 1],
                        )
                for h in range(2):
                    for jt in sets[g]:
                        nc.tensor.matmul(
                            oT[h][:], fcY[:, jt * 128 : (jt + 1) * 128],
                            ATC[:, jt * 1024 + h * HALF : jt * 1024 + (h + 1) * HALF],
                            start=(g == 0 and jt == sets[0][0]),
                            stop=(g == 2 and jt == sets[2][-1]),
                        )
                    if g == 2:
                        # epilogue for this half overlaps the other half
                        osb = epip.tile([128, HALF], F32, tag="osb")
                        nc.vector.tensor_tensor(
                            out=osb[:], in0=oT[h][:], in1=dbc[h][:],
                            op=mybir.AluOpType.mult,
                        )
                        nc.sync.dma_start(
                            out=outT[:, h * HALF : (h + 1) * HALF], in_=osb[:]
                        )

    nc.compile()
    return nc


def kernel(values, adjacency, W, b):
    from concourse.bass_utils import run_bass_kernel_spmd

    if "nc" not in _CACHE:
        _CACHE["nc"] = _build()
    nc = _CACHE["nc"]

    values = np.asarray(values, dtype=np.float32)
    adjacency = np.asarray(adjacency, dtype=np.float32)
    W = np.asarray(W, dtype=np.float32)
    b = np.asarray(b, dtype=np.float32)

    vt = np.ascontiguousarray(values.T)                  # [D, N]
    bb = np.ascontiguousarray(np.tile(b[None, :], (128, 1)))
    ident = np.eye(128, dtype=np.float32)

    def interleave(block, nst, R, w):
        # block: A rows [nst*R*128/..., hmm] -> see at_ph declaration
        return np.ascontiguousarray(
            block.T.reshape(nst, R, 128, w).transpose(0, 2, 1, 3).reshape(nst * 128, R * w)
        )

    in_maps = []
    for k in range(N_CORES):
        blk = adjacency[k * ROWS : (k + 1) * ROWS]       # [1024, 8192]
        m = {
            "vt": vt, "w": W, "bb": bb, "ident": ident,
        }
        for ph in range(3):
            lo, w_, R, nst = PH_LO[ph], PH_W[ph], PH_R[ph], PH_NST[ph]
            m[f"at{ph}"] = interleave(blk[lo : lo + w_, :], nst, R, w_)
        in_maps.append(m)
    trace = bool(int(os.environ.get("GCN_TRACE", "0")))
    res = run_bass_kernel_spmd(nc, in_maps, list(range(N_CORES)), trace=trace)
    if trace and res.exec_time_ns is not None:
        print(f"HW exec time: {res.exec_time_ns} ns")
        _CACHE["exec_time_ns"] = res.exec_time_ns
    out = np.concatenate(
        [res.results[k]["outT"].T for k in range(N_CORES)], axis=0
    ).astype(np.float32)
    return out
